# revision 5
# baseline (speedup 1.0000x reference)
"""EnergySNN single-step kernel for Trainium2, 8-core data parallel.

Reference computation (per batch row, D=512, L=3 layers):
    s = 0.5*x
    for i in 0..2:
        fb_in = spikes_h[i+1]            (i<2)   |  readout/||readout||  (i==2)
        ff = s @ W_ff[i].T + b_ff[i]
        fb = fb_in @ W_fb[i].T + b_fb[i]
        a_new = 0.9*dend[i] + 0.1*(ff+fb)
        sm    = 0.9*soma[i]*(1-spikes_h[i]) + 0.1*a_new
        bb    = 0.96*b[i] + 0.04*spikes_h[i]
        spk   = (sm - (0.1 + 1.8*bb)) > 0
        s = spk
    readout_new = 0.9*readout + s @ W_out.T + b_out
    out = [sm(3), spk(3), a_new(3), bb(3), readout_new(1)]  -> [13, B, D]

Strategy: pure data parallel over batch (8192 -> 8 x 1024), transposed
[D, B_local] device layout so matmul rhs (contraction over D on partitions)
and elementwise state updates share one layout.

All device I/O is 16-bit: activations/rhs in bf16, states in fp16, weights in
single bf16 (no exact splits). Elementwise algebra is folded on the host into
three fused state planes so the device does only:
    psum = dendm*I + 0.1*(ff+fb)          (identity matmul folds the dendrite)
    a_new = copy(psum)                     (scalar engine, f16 out)
    sm    = 0.1*psum + somam               (one DVE op; somam=0.9*soma*(1-s))
    spk   = (13.889*sm) > tbps             (one DVE op; tbps=(0.1+1.728b)/0.072+s)
The bb output plane (0.96*b + 0.04*s) needs no matmul and is computed on the
host. This cuts HBM traffic ~2.1x and PE time ~3.4x vs the exact-split
baseline at a total relative error of ~6e-3 (tolerance 2e-2).

All load-side arrays are pre-swizzled on the host into the exact SBUF tile
layout ([128 partitions, chunks*freedim], contiguous per partition) so every
load DMA is a plain 2D linear copy (large merged packets, cheap descriptor
generation). Loads are issued from three engine queues in parallel
(sync/vector/scalar, in consumption order) to hide issue latency at startup;
each store is issued from an otherwise-idle queue gated on its producer.
"""

import numpy as np
import sys

sys.path.insert(0, "/opt/trn_rl_repo")

import concourse.bass as bass
import concourse.bacc as bacc
import concourse.mybir as mybir
from concourse import tile
from concourse.bass_utils import run_bass_kernel_spmd

F32 = mybir.dt.float32
BF16 = mybir.dt.bfloat16
F16 = mybir.dt.float16
NP_BF16 = mybir.dt.np(BF16)
OP = mybir.AluOpType
AF = mybir.ActivationFunctionType

# Problem constants (hardcoded per contract)
B = 8192
D = 512
L = 3
NCORES = 8
BL = B // NCORES          # 1024 batch rows per core
P = 128                   # partitions
KC = D // P               # 4 contraction chunks
MC = D // P               # 4 output-d chunks
NW = BL                   # full local batch as one free-dim tile
NH = 512                  # matmul free-dim (one PSUM bank of f32)
NCH = NW // NH            # 2 matmul half-groups per psum tile

ALPHA_M = np.float32(0.9)
ALPHA_A = np.float32(0.9)
RHO = np.float32(0.96)
BETA = np.float32(1.8)
B0 = np.float32(0.1)
ALPHA_OUT = np.float32(0.9)
EPS = np.float32(1e-12)
ONE_MINUS_AM = np.float32(0.1)
ONE_MINUS_AA = np.float32(0.1)
SC = np.float32(1.0) / (BETA * (np.float32(1.0) - RHO))   # 1/0.072

OUTPUT_NAMES = ["outSmT", "outAnT", "outSpkT", "outRdT"]


def build_program(use_bias=False):
    """Build the per-core SPMD Bass/Tile program."""
    nc = bacc.Bacc("TRN2", target_bir_lowering=False)

    # --- DRAM I/O (per-core shapes, host-preswizzled tile layouts) ---
    xT = nc.dram_tensor("xT", [P, KC * NW], BF16, kind="ExternalInput")
    spk1T = nc.dram_tensor("spk1T", [P, KC * NW], BF16, kind="ExternalInput")
    spk2T = nc.dram_tensor("spk2T", [P, KC * NW], BF16, kind="ExternalInput")
    fbnT = nc.dram_tensor("fbnT", [P, KC * NW], BF16, kind="ExternalInput")
    readT = nc.dram_tensor("readT", [P, MC * NW], F16, kind="ExternalInput")
    somamT = nc.dram_tensor("somamT", [L, P, MC * NW], F16, kind="ExternalInput")
    dendmT = nc.dram_tensor("dendmT", [L, P, MC * NW], BF16, kind="ExternalInput")
    tbpsT = nc.dram_tensor("tbpsT", [L, P, MC * NW], F16, kind="ExternalInput")
    wffT = nc.dram_tensor("wffT", [L, P, KC * D], BF16, kind="ExternalInput")
    wfbT = nc.dram_tensor("wfbT", [L, P, KC * D], BF16, kind="ExternalInput")
    woutT = nc.dram_tensor("woutT", [P, KC * D], BF16, kind="ExternalInput")
    identT = nc.dram_tensor("identT", [P, P], BF16, kind="ExternalInput")
    bcomb = nc.dram_tensor("bcomb", [L, 1, D], BF16, kind="ExternalInput")
    boutD = nc.dram_tensor("boutD", [1, D], BF16, kind="ExternalInput")
    # outputs (transposed [D, BL] world)
    outSmT = nc.dram_tensor("outSmT", [L, D, BL], F16, kind="ExternalOutput")
    outAnT = nc.dram_tensor("outAnT", [L, D, BL], F16, kind="ExternalOutput")
    outSpkT = nc.dram_tensor("outSpkT", [L, D, BL], BF16, kind="ExternalOutput")
    outRdT = nc.dram_tensor("outRdT", [D, BL], F16, kind="ExternalOutput")

    with tile.TileContext(nc) as tc:
        with (
            tc.tile_pool(name="wpool", bufs=1) as wp,
            tc.tile_pool(name="spool", bufs=1) as sp,
            tc.tile_pool(name="ppool", bufs=1, space=bass.MemorySpace.PSUM) as pp,
        ):
            ident = wp.tile([P, P], BF16, tag="ident")
            nc.sync.dma_start(ident[:], identT[:, :])
            if use_bias:
                onesN = wp.tile([1, NH], BF16, tag="onesN")
                nc.vector.memset(onesN[:], 1.0)
                bc_sb = [wp.tile([1, D], BF16, tag=f"bc{i}", name=f"bc{i}")
                         for i in range(L)]
                bo_sb = wp.tile([1, D], BF16, tag="bo")
                for i in range(L):
                    nc.scalar.dma_start(bc_sb[i][:], bcomb[i, :, :])
                nc.scalar.dma_start(bo_sb[:], boutD[:, :])

            def wload(eng, name, src2d):
                t = wp.tile([P, KC * D], BF16, tag="w", bufs=7, name=name)
                eng.dma_start(t[:], src2d)
                return t

            def wsl(t, k, m):
                # lhsT [P, 128] for contraction chunk k, output chunk m
                return t[:, k * D + m * P: k * D + (m + 1) * P]

            def pload(eng, name, src2d, dt, tag, nb):
                t = sp.tile([P, MC * NW], dt, tag=tag, bufs=nb, name=name)
                eng.dma_start(t[:], src2d)
                return t

            # ---- prologue loads: three queues in consumption order ----
            # Two balanced load queues in consumption order (DMA queues
            # fair-share HBM by packet count, so late-needed data must not
            # sit ahead of early-needed data). All stores go on scalar.
            # sync:   ident, w0ff, x, dendm0, w1ff, s2, dendm1, w2ff, fbn, dendm2, wout
            # gpsimd: w0fb, s1, somam0, tbps0, w1fb, somam1, tbps1, w2fb, somam2, tbps2, read
            w0ff = wload(nc.sync, "w0ff", wffT[0])
            w0fb = wload(nc.gpsimd, "w0fb", wfbT[0])
            x_sb = pload(nc.sync, "x_sb", xT[:, :], BF16, "rx", 1)
            s1_sb = pload(nc.gpsimd, "s1_sb", spk1T[:, :], BF16, "rs1", 1)
            dendm0 = pload(nc.sync, "dendm0", dendmT[0], BF16, "dendm", 3)
            somam0 = pload(nc.gpsimd, "somam0", somamT[0], F16, "somam", 3)
            tbps0 = pload(nc.gpsimd, "tbps0", tbpsT[0], F16, "tbps", 3)
            w1ff = wload(nc.sync, "w1ff", wffT[1])
            w1fb = wload(nc.gpsimd, "w1fb", wfbT[1])
            s2_sb = pload(nc.sync, "s2_sb", spk2T[:, :], BF16, "rs2", 1)
            dendm1 = pload(nc.sync, "dendm1", dendmT[1], BF16, "dendm", 3)
            somam1 = pload(nc.gpsimd, "somam1", somamT[1], F16, "somam", 3)
            tbps1 = pload(nc.gpsimd, "tbps1", tbpsT[1], F16, "tbps", 3)
            w2ff = wload(nc.sync, "w2ff", wffT[2])
            w2fb = wload(nc.gpsimd, "w2fb", wfbT[2])
            fbn_sb = pload(nc.sync, "fbn_sb", fbnT[:, :], BF16, "rfbn", 1)
            dendm2 = pload(nc.sync, "dendm2", dendmT[2], BF16, "dendm", 3)
            somam2 = pload(nc.gpsimd, "somam2", somamT[2], F16, "somam", 3)
            tbps2 = pload(nc.gpsimd, "tbps2", tbpsT[2], F16, "tbps", 3)
            read_sb = pload(nc.gpsimd, "read_sb", readT[:, :], F16, "read", 1)
            wout_sb = wload(nc.sync, "wout", woutT[:, :])

            wgt = {0: (w0ff, w0fb), 1: (w1ff, w1fb), 2: (w2ff, w2fb)}
            stt = {0: (dendm0, somam0, tbps0), 1: (dendm1, somam1, tbps1),
                   2: (dendm2, somam2, tbps2)}
            rhs_fb_t = {0: s1_sb, 1: s2_sb, 2: fbn_sb}

            # rhs slices: plane tile t, chunk k, half n -> [128, 512]
            def rsl(t, k, n):
                return t[:, k * NW + n * NH: k * NW + n * NH + NH]

            # ---- layer loop ----
            rhs_ff = [[rsl(x_sb, k, n) for n in range(NCH)] for k in range(KC)]
            for i in range(L):
                ff_w, fb_w = wgt[i]
                dm, so, tb = stt[i]
                fb_t = rhs_fb_t[i]
                new_spk = []
                for m in range(MC):
                    ps = pp.tile([P, NW], F32, tag="mm", bufs=3)
                    for n in range(NCH):
                        psn = ps[:, n * NH:(n + 1) * NH]
                        dsl = dm[:, m * NW + n * NH: m * NW + n * NH + NH]
                        mm = [(ident[:], dsl)]
                        for k in range(KC):
                            mm.append((wsl(fb_w, k, m), rsl(fb_t, k, n)))
                        for k in range(KC):
                            mm.append((wsl(ff_w, k, m), rhs_ff[k][n]))
                        for j, (lw, rr) in enumerate(mm):
                            last = (j == len(mm) - 1) and not use_bias
                            nc.tensor.matmul(psn, lw, rr, start=(j == 0),
                                             stop=last)
                        if use_bias:
                            nc.tensor.matmul(psn, bc_sb[i][0:1, m * P:(m + 1) * P],
                                             onesN[0:1, :], start=False, stop=True)
                    msl = slice(m * NW, (m + 1) * NW)
                    anew = sp.tile([P, NW], F16, tag="anew", bufs=4)
                    nc.scalar.activation(anew[:], ps[:], AF.Copy)
                    sm = sp.tile([P, NW], F16, tag="sm", bufs=4)
                    nc.vector.scalar_tensor_tensor(
                        sm[:], ps[:], float(ONE_MINUS_AM), so[:, msl],
                        OP.mult, OP.add)
                    spk = sp.tile([P, NW], BF16, tag="spk", bufs=9)
                    nc.vector.scalar_tensor_tensor(
                        spk[:], sm[:], float(SC), tb[:, msl], OP.mult, OP.is_gt)
                    dsl2 = slice(m * P, (m + 1) * P)
                    nc.scalar.dma_start(outAnT[i, dsl2, :], anew[:])
                    nc.scalar.dma_start(outSmT[i, dsl2, :], sm[:])
                    nc.scalar.dma_start(outSpkT[i, dsl2, :], spk[:])
                    new_spk.append(spk)
                rhs_ff = [[new_spk[k][:, n * NH:(n + 1) * NH]
                           for n in range(NCH)] for k in range(KC)]

            # ---- readout: 0.9*readout + spk2 @ W_out.T + b_out ----
            for m in range(MC):
                psr = pp.tile([P, NW], F32, tag="mm", bufs=3)
                for n in range(NCH):
                    psn = psr[:, n * NH:(n + 1) * NH]
                    for k in range(KC):
                        last = (k == KC - 1) and not use_bias
                        nc.tensor.matmul(psn, wsl(wout_sb, k, m), rhs_ff[k][n],
                                         start=(k == 0), stop=last)
                    if use_bias:
                        nc.tensor.matmul(psn, bo_sb[0:1, m * P:(m + 1) * P],
                                         onesN[0:1, :], start=False, stop=True)
                rn = sp.tile([P, NW], F16, tag="rn", bufs=2)
                nc.vector.scalar_tensor_tensor(
                    rn[:], read_sb[:, m * NW:(m + 1) * NW], float(ALPHA_OUT),
                    psr[:], OP.mult, OP.add)
                nc.scalar.dma_start(outRdT[m * P:(m + 1) * P, :], rn[:])

    nc.compile()
    return nc


def _swz(a2d):
    """[D, X] -> [P, (D//P)*X] SBUF tile layout, contiguous per partition."""
    Dd, X = a2d.shape
    k = Dd // P
    return np.ascontiguousarray(
        a2d.reshape(k, P, X).transpose(1, 0, 2).reshape(P, k * X))


def make_in_maps(x, soma, spikes_h, dendrites, b, readout,
                 W_ff, b_ff, W_fb, b_fb, W_out, b_out):
    """Shard + transpose inputs; fold elementwise algebra into fused planes."""
    f32 = np.float32
    f16 = np.float16
    x = np.asarray(x, f32)
    soma = np.asarray(soma, f32)
    spikes_h = np.asarray(spikes_h, f32)
    dendrites = np.asarray(dendrites, f32)
    b = np.asarray(b, f32)
    readout = np.asarray(readout, f32)
    W_ff = np.asarray(W_ff, f32)
    b_ff = np.asarray(b_ff, f32)
    W_fb = np.asarray(W_fb, f32)
    b_fb = np.asarray(b_fb, f32)
    W_out = np.asarray(W_out, f32)
    b_out = np.asarray(b_out, f32)

    # weights, transposed, with 0.1 (and 0.5 input scale for layer 0) folded
    wffT = np.stack([_swz(
        (((ONE_MINUS_AA * (f32(0.5) if i == 0 else f32(1.0))) * W_ff[i]).T
         ).astype(NP_BF16)) for i in range(L)])
    wfbT = np.stack([_swz(((ONE_MINUS_AA * W_fb[i]).T).astype(NP_BF16))
                     for i in range(L)])
    woutT = _swz(W_out.T.astype(NP_BF16))
    identA = np.eye(P, dtype=f32).astype(NP_BF16)
    bcombA = (ONE_MINUS_AA * (b_ff + b_fb)).reshape(L, 1, D).astype(NP_BF16)
    boutA = b_out.reshape(1, D).astype(NP_BF16)

    # fused state planes (full batch, then shard)
    somam = (ALPHA_M * soma * (f32(1.0) - spikes_h)).astype(f16)
    dendm = (ALPHA_A * dendrites).astype(NP_BF16)
    tbps = (SC * (B0 + BETA * RHO * b) + spikes_h).astype(f16)
    nrm = np.maximum(np.sqrt(np.sum(readout * readout, axis=1, keepdims=True)),
                     EPS).astype(f32)
    fbn = (readout / nrm).astype(NP_BF16)
    x16 = x.astype(NP_BF16)
    spk16 = spikes_h.astype(NP_BF16)
    read16 = readout.astype(f16)

    in_maps = []
    for c in range(NCORES):
        sl = slice(c * BL, (c + 1) * BL)
        in_maps.append({
            "xT": _swz(x16[sl].T),
            "spk1T": _swz(spk16[1, sl].T),
            "spk2T": _swz(spk16[2, sl].T),
            "fbnT": _swz(fbn[sl].T),
            "readT": _swz(read16[sl].T),
            "somamT": np.stack([_swz(somam[i, sl].T) for i in range(L)]),
            "dendmT": np.stack([_swz(dendm[i, sl].T) for i in range(L)]),
            "tbpsT": np.stack([_swz(tbps[i, sl].T) for i in range(L)]),
            "wffT": wffT,
            "wfbT": wfbT,
            "woutT": woutT,
            "identT": identA,
            "bcomb": bcombA,
            "boutD": boutA,
        })
    return in_maps


def host_bb(b, spikes_h):
    """bb = 0.96*b + 0.04*s needs no matmul -> exact f32 on the host."""
    return (RHO * np.asarray(b, np.float32)
            + (np.float32(1.0) - RHO) * np.asarray(spikes_h, np.float32))


def assemble_output(results, bb=None):
    """per-core 16-bit outputs -> [13, B, D] f32 (+ host bb planes)."""
    out = np.empty((4 * L + 1, B, D), np.float32)
    for c in range(NCORES):
        sl = slice(c * BL, (c + 1) * BL)
        r = results[c]
        for i in range(L):
            out[i, sl, :] = r["outSmT"][i].astype(np.float32).T
            out[L + i, sl, :] = r["outSpkT"][i].astype(np.float32).T
            out[2 * L + i, sl, :] = r["outAnT"][i].astype(np.float32).T
        out[4 * L, sl, :] = r["outRdT"].astype(np.float32).T
    if bb is not None:
        out[3 * L:4 * L] = bb
    return out


_CACHE = {}


def _get_program(use_bias=False):
    key = ("nc", use_bias)
    if key not in _CACHE:
        _CACHE[key] = build_program(use_bias)
    return _CACHE[key]


def kernel(**inputs):
    use_bias = bool(np.any(inputs["b_ff"]) or np.any(inputs["b_fb"])
                    or np.any(inputs["b_out"]))
    nc = _get_program(use_bias)
    in_maps = make_in_maps(**inputs)
    bb = host_bb(inputs["b"], inputs["spikes_h"])
    res = run_bass_kernel_spmd(nc, in_maps, core_ids=list(range(NCORES)))
    return assemble_output(res.results, bb)


# revision 6
# speedup vs baseline: 1.3145x; 1.3145x over previous
"""EnergySNN single-step kernel for Trainium2, 8-core data parallel.

Reference computation (per batch row, D=512, L=3 layers):
    s = 0.5*x
    for i in 0..2:
        fb_in = spikes_h[i+1]            (i<2)   |  readout/||readout||  (i==2)
        ff = s @ W_ff[i].T + b_ff[i]
        fb = fb_in @ W_fb[i].T + b_fb[i]
        a_new = 0.9*dend[i] + 0.1*(ff+fb)
        sm    = 0.9*soma[i]*(1-spikes_h[i]) + 0.1*a_new
        bb    = 0.96*b[i] + 0.04*spikes_h[i]
        spk   = (sm - (0.1 + 1.8*bb)) > 0
        s = spk
    readout_new = 0.9*readout + s @ W_out.T + b_out
    out = [sm(3), spk(3), a_new(3), bb(3), readout_new(1)]  -> [13, B, D]

Strategy: pure data parallel over batch (8192 -> 8 x 1024), transposed
[D, B_local] device layout so matmul rhs (contraction over D on partitions)
and elementwise state updates share one layout.

All device I/O is 16-bit or fp8: rhs planes (x, spike history, normalized
readout) and the spike outputs are fp8e4m3 (spikes are exactly 0/1 in fp8),
states are fp16/bf16, weights single bf16 (no exact splits). Elementwise algebra is folded on the host into
three fused state planes so the device does only:
    psum = dendm*I + 0.1*(ff+fb)          (identity matmul folds the dendrite)
    a_new = copy(psum)                     (scalar engine, f16 out)
    sm    = 0.1*psum + somam               (one DVE op; somam=0.9*soma*(1-s))
    spk   = (13.889*sm) > tbps             (one DVE op; tbps=(0.1+1.728b)/0.072+s)
The bb output plane (0.96*b + 0.04*s) needs no matmul and is computed on the
host. This cuts HBM traffic ~2.1x and PE time ~3.4x vs the exact-split
baseline at a total relative error of ~6e-3 (tolerance 2e-2).

All load-side arrays are pre-swizzled on the host into the exact SBUF tile
layout ([128 partitions, chunks*freedim], contiguous per partition) so every
load DMA is a plain 2D linear copy (large merged packets, cheap descriptor
generation). Loads are issued from three engine queues in parallel
(sync/vector/scalar, in consumption order) to hide issue latency at startup;
each store is issued from an otherwise-idle queue gated on its producer.
"""

import numpy as np
import sys

sys.path.insert(0, "/opt/trn_rl_repo")

import concourse.bass as bass
import concourse.bacc as bacc
import concourse.mybir as mybir
from concourse import tile
from concourse.bass_utils import run_bass_kernel_spmd

F32 = mybir.dt.float32
BF16 = mybir.dt.bfloat16
F16 = mybir.dt.float16
F8 = mybir.dt.float8e4
NP_F8 = mybir.dt.np(F8)
NP_BF16 = mybir.dt.np(BF16)
OP = mybir.AluOpType
AF = mybir.ActivationFunctionType

# Problem constants (hardcoded per contract)
B = 8192
D = 512
L = 3
NCORES = 8
BL = B // NCORES          # 1024 batch rows per core
P = 128                   # partitions
KC = D // P               # 4 contraction chunks
MC = D // P               # 4 output-d chunks
NW = BL                   # full local batch as one free-dim tile
NH = 512                  # matmul free-dim (one PSUM bank of f32)
NCH = NW // NH            # 2 matmul half-groups per psum tile

ALPHA_M = np.float32(0.9)
ALPHA_A = np.float32(0.9)
RHO = np.float32(0.96)
BETA = np.float32(1.8)
B0 = np.float32(0.1)
ALPHA_OUT = np.float32(0.9)
EPS = np.float32(1e-12)
ONE_MINUS_AM = np.float32(0.1)
ONE_MINUS_AA = np.float32(0.1)
SC = np.float32(1.0) / (BETA * (np.float32(1.0) - RHO))   # 1/0.072

OUTPUT_NAMES = ["outSmT", "outAnT", "outSpkT", "outRdT"]


def build_program(use_bias=False):
    """Build the per-core SPMD Bass/Tile program."""
    nc = bacc.Bacc("TRN2", target_bir_lowering=False)

    # --- DRAM I/O (per-core shapes, host-preswizzled tile layouts) ---
    xT = nc.dram_tensor("xT", [P, KC * NW], F8, kind="ExternalInput")
    spk1T = nc.dram_tensor("spk1T", [P, KC * NW], F8, kind="ExternalInput")
    spk2T = nc.dram_tensor("spk2T", [P, KC * NW], F8, kind="ExternalInput")
    fbnT = nc.dram_tensor("fbnT", [P, KC * NW], F8, kind="ExternalInput")
    readT = nc.dram_tensor("readT", [P, MC * NW], F16, kind="ExternalInput")
    somamT = nc.dram_tensor("somamT", [L, P, MC * NW], F16, kind="ExternalInput")
    dendmT = nc.dram_tensor("dendmT", [L, P, MC * NW], BF16, kind="ExternalInput")
    tbpsT = nc.dram_tensor("tbpsT", [L, P, MC * NW], F16, kind="ExternalInput")
    wffT = nc.dram_tensor("wffT", [L, P, KC * D], BF16, kind="ExternalInput")
    wfbT = nc.dram_tensor("wfbT", [L, P, KC * D], BF16, kind="ExternalInput")
    woutT = nc.dram_tensor("woutT", [P, KC * D], BF16, kind="ExternalInput")
    identT = nc.dram_tensor("identT", [P, P], BF16, kind="ExternalInput")
    bcomb = nc.dram_tensor("bcomb", [L, 1, D], BF16, kind="ExternalInput")
    boutD = nc.dram_tensor("boutD", [1, D], BF16, kind="ExternalInput")
    # outputs (transposed [D, BL] world)
    outSmT = nc.dram_tensor("outSmT", [L, D, BL], F16, kind="ExternalOutput")
    outAnT = nc.dram_tensor("outAnT", [L, D, BL], F16, kind="ExternalOutput")
    outSpkT = nc.dram_tensor("outSpkT", [L, D, BL], F8, kind="ExternalOutput")
    outRdT = nc.dram_tensor("outRdT", [D, BL], F16, kind="ExternalOutput")

    with tile.TileContext(nc) as tc:
        with (
            tc.tile_pool(name="wpool", bufs=1) as wp,
            tc.tile_pool(name="spool", bufs=1) as sp,
            tc.tile_pool(name="ppool", bufs=1, space=bass.MemorySpace.PSUM) as pp,
        ):
            ident = wp.tile([P, P], BF16, tag="ident")
            nc.sync.dma_start(ident[:], identT[:, :])
            if use_bias:
                onesN = wp.tile([1, NH], BF16, tag="onesN")
                nc.vector.memset(onesN[:], 1.0)
                bc_sb = [wp.tile([1, D], BF16, tag=f"bc{i}", name=f"bc{i}")
                         for i in range(L)]
                bo_sb = wp.tile([1, D], BF16, tag="bo")
                for i in range(L):
                    nc.scalar.dma_start(bc_sb[i][:], bcomb[i, :, :])
                nc.scalar.dma_start(bo_sb[:], boutD[:, :])

            def wload(eng, name, src2d):
                t = wp.tile([P, KC * D], BF16, tag="w", bufs=7, name=name)
                eng.dma_start(t[:], src2d)
                return t

            def wsl(t, k, m):
                # lhsT [P, 128] for contraction chunk k, output chunk m
                return t[:, k * D + m * P: k * D + (m + 1) * P]

            def pload(eng, name, src2d, dt, tag, nb):
                t = sp.tile([P, MC * NW], dt, tag=tag, bufs=nb, name=name)
                eng.dma_start(t[:], src2d)
                return t

            # ---- prologue loads: three queues in consumption order ----
            # All loads on the sync HWDGE queue in strict consumption
            # order -- DMA queues fair-share HBM by packet count, so a single
            # ordered queue is the only real priority mechanism. All stores go
            # on the scalar HWDGE queue (gpsimd DMA is the slow software-DGE
            # path; never use it).
            w0ff = wload(nc.sync, "w0ff", wffT[0])
            w0fb = wload(nc.sync, "w0fb", wfbT[0])
            x_sb = pload(nc.sync, "x_sb", xT[:, :], F8, "rx", 1)
            s1_sb = pload(nc.sync, "s1_sb", spk1T[:, :], F8, "rs1", 1)
            dendm0 = pload(nc.sync, "dendm0", dendmT[0], BF16, "dendm", 3)
            somam0 = pload(nc.sync, "somam0", somamT[0], F16, "somam", 3)
            tbps0 = pload(nc.sync, "tbps0", tbpsT[0], F16, "tbps", 3)
            w1ff = wload(nc.sync, "w1ff", wffT[1])
            w1fb = wload(nc.sync, "w1fb", wfbT[1])
            s2_sb = pload(nc.sync, "s2_sb", spk2T[:, :], F8, "rs2", 1)
            dendm1 = pload(nc.sync, "dendm1", dendmT[1], BF16, "dendm", 3)
            somam1 = pload(nc.sync, "somam1", somamT[1], F16, "somam", 3)
            tbps1 = pload(nc.sync, "tbps1", tbpsT[1], F16, "tbps", 3)
            w2ff = wload(nc.sync, "w2ff", wffT[2])
            w2fb = wload(nc.sync, "w2fb", wfbT[2])
            fbn_sb = pload(nc.sync, "fbn_sb", fbnT[:, :], F8, "rfbn", 1)
            dendm2 = pload(nc.sync, "dendm2", dendmT[2], BF16, "dendm", 3)
            somam2 = pload(nc.sync, "somam2", somamT[2], F16, "somam", 3)
            tbps2 = pload(nc.sync, "tbps2", tbpsT[2], F16, "tbps", 3)
            read_sb = pload(nc.sync, "read_sb", readT[:, :], F16, "read", 1)
            wout_sb = wload(nc.sync, "wout", woutT[:, :])

            wgt = {0: (w0ff, w0fb), 1: (w1ff, w1fb), 2: (w2ff, w2fb)}
            stt = {0: (dendm0, somam0, tbps0), 1: (dendm1, somam1, tbps1),
                   2: (dendm2, somam2, tbps2)}
            rhs_fb_t = {0: s1_sb, 1: s2_sb, 2: fbn_sb}

            # rhs slices: plane tile t, chunk k, half n -> [128, 512]
            def rsl(t, k, n):
                return t[:, k * NW + n * NH: k * NW + n * NH + NH]

            # ---- layer loop ----
            rhs_ff = [[rsl(x_sb, k, n) for n in range(NCH)] for k in range(KC)]
            for i in range(L):
                ff_w, fb_w = wgt[i]
                dm, so, tb = stt[i]
                fb_t = rhs_fb_t[i]
                new_spk = []
                for m in range(MC):
                    ps = pp.tile([P, NW], F32, tag="mm", bufs=3)
                    for n in range(NCH):
                        psn = ps[:, n * NH:(n + 1) * NH]
                        dsl = dm[:, m * NW + n * NH: m * NW + n * NH + NH]
                        mm = [(ident[:], dsl)]
                        for k in range(KC):
                            mm.append((wsl(fb_w, k, m), rsl(fb_t, k, n)))
                        for k in range(KC):
                            mm.append((wsl(ff_w, k, m), rhs_ff[k][n]))
                        for j, (lw, rr) in enumerate(mm):
                            last = (j == len(mm) - 1) and not use_bias
                            nc.tensor.matmul(psn, lw, rr, start=(j == 0),
                                             stop=last)
                        if use_bias:
                            nc.tensor.matmul(psn, bc_sb[i][0:1, m * P:(m + 1) * P],
                                             onesN[0:1, :], start=False, stop=True)
                    msl = slice(m * NW, (m + 1) * NW)
                    anew = sp.tile([P, NW], F16, tag="anew", bufs=4)
                    nc.scalar.activation(anew[:], ps[:], AF.Copy)
                    sm = sp.tile([P, NW], F16, tag="sm", bufs=4)
                    nc.vector.scalar_tensor_tensor(
                        sm[:], ps[:], float(ONE_MINUS_AM), so[:, msl],
                        OP.mult, OP.add)
                    spk = sp.tile([P, NW], F8, tag="spk", bufs=9)
                    nc.vector.scalar_tensor_tensor(
                        spk[:], sm[:], float(SC), tb[:, msl], OP.mult, OP.is_gt)
                    dsl2 = slice(m * P, (m + 1) * P)
                    nc.scalar.dma_start(outAnT[i, dsl2, :], anew[:])
                    nc.scalar.dma_start(outSmT[i, dsl2, :], sm[:])
                    nc.scalar.dma_start(outSpkT[i, dsl2, :], spk[:])
                    new_spk.append(spk)
                rhs_ff = [[new_spk[k][:, n * NH:(n + 1) * NH]
                           for n in range(NCH)] for k in range(KC)]

            # ---- readout: 0.9*readout + spk2 @ W_out.T + b_out ----
            for m in range(MC):
                psr = pp.tile([P, NW], F32, tag="mm", bufs=3)
                for n in range(NCH):
                    psn = psr[:, n * NH:(n + 1) * NH]
                    for k in range(KC):
                        last = (k == KC - 1) and not use_bias
                        nc.tensor.matmul(psn, wsl(wout_sb, k, m), rhs_ff[k][n],
                                         start=(k == 0), stop=last)
                    if use_bias:
                        nc.tensor.matmul(psn, bo_sb[0:1, m * P:(m + 1) * P],
                                         onesN[0:1, :], start=False, stop=True)
                rn = sp.tile([P, NW], F16, tag="rn", bufs=2)
                nc.vector.scalar_tensor_tensor(
                    rn[:], read_sb[:, m * NW:(m + 1) * NW], float(ALPHA_OUT),
                    psr[:], OP.mult, OP.add)
                nc.scalar.dma_start(outRdT[m * P:(m + 1) * P, :], rn[:])

    nc.compile()
    return nc


def _swz(a2d):
    """[D, X] -> [P, (D//P)*X] SBUF tile layout, contiguous per partition."""
    Dd, X = a2d.shape
    k = Dd // P
    return np.ascontiguousarray(
        a2d.reshape(k, P, X).transpose(1, 0, 2).reshape(P, k * X))


def make_in_maps(x, soma, spikes_h, dendrites, b, readout,
                 W_ff, b_ff, W_fb, b_fb, W_out, b_out):
    """Shard + transpose inputs; fold elementwise algebra into fused planes."""
    f32 = np.float32
    f16 = np.float16
    x = np.asarray(x, f32)
    soma = np.asarray(soma, f32)
    spikes_h = np.asarray(spikes_h, f32)
    dendrites = np.asarray(dendrites, f32)
    b = np.asarray(b, f32)
    readout = np.asarray(readout, f32)
    W_ff = np.asarray(W_ff, f32)
    b_ff = np.asarray(b_ff, f32)
    W_fb = np.asarray(W_fb, f32)
    b_fb = np.asarray(b_fb, f32)
    W_out = np.asarray(W_out, f32)
    b_out = np.asarray(b_out, f32)

    # weights, transposed, with 0.1 (and 0.5 input scale for layer 0) folded
    wffT = np.stack([_swz(
        (((ONE_MINUS_AA * (f32(0.5) if i == 0 else f32(1.0))) * W_ff[i]).T
         ).astype(NP_BF16)) for i in range(L)])
    wfbT = np.stack([_swz(((ONE_MINUS_AA * W_fb[i]).T).astype(NP_BF16))
                     for i in range(L)])
    woutT = _swz(W_out.T.astype(NP_BF16))
    identA = np.eye(P, dtype=f32).astype(NP_BF16)
    bcombA = (ONE_MINUS_AA * (b_ff + b_fb)).reshape(L, 1, D).astype(NP_BF16)
    boutA = b_out.reshape(1, D).astype(NP_BF16)

    # fused state planes (full batch, then shard)
    somam = (ALPHA_M * soma * (f32(1.0) - spikes_h)).astype(f16)
    dendm = (ALPHA_A * dendrites).astype(NP_BF16)
    tbps = (SC * (B0 + BETA * RHO * b) + spikes_h).astype(f16)
    nrm = np.maximum(np.sqrt(np.sum(readout * readout, axis=1, keepdims=True)),
                     EPS).astype(f32)
    fbn = (readout / nrm).astype(NP_F8)
    x16 = x.astype(NP_F8)
    spk16 = spikes_h.astype(NP_F8)
    read16 = readout.astype(f16)

    in_maps = []
    for c in range(NCORES):
        sl = slice(c * BL, (c + 1) * BL)
        in_maps.append({
            "xT": _swz(x16[sl].T),
            "spk1T": _swz(spk16[1, sl].T),
            "spk2T": _swz(spk16[2, sl].T),
            "fbnT": _swz(fbn[sl].T),
            "readT": _swz(read16[sl].T),
            "somamT": np.stack([_swz(somam[i, sl].T) for i in range(L)]),
            "dendmT": np.stack([_swz(dendm[i, sl].T) for i in range(L)]),
            "tbpsT": np.stack([_swz(tbps[i, sl].T) for i in range(L)]),
            "wffT": wffT,
            "wfbT": wfbT,
            "woutT": woutT,
            "identT": identA,
            "bcomb": bcombA,
            "boutD": boutA,
        })
    return in_maps


def host_bb(b, spikes_h):
    """bb = 0.96*b + 0.04*s needs no matmul -> exact f32 on the host."""
    return (RHO * np.asarray(b, np.float32)
            + (np.float32(1.0) - RHO) * np.asarray(spikes_h, np.float32))


def assemble_output(results, bb=None):
    """per-core 16-bit outputs -> [13, B, D] f32 (+ host bb planes)."""
    out = np.empty((4 * L + 1, B, D), np.float32)
    for c in range(NCORES):
        sl = slice(c * BL, (c + 1) * BL)
        r = results[c]
        for i in range(L):
            out[i, sl, :] = r["outSmT"][i].astype(np.float32).T
            out[L + i, sl, :] = r["outSpkT"][i].astype(np.float32).T
            out[2 * L + i, sl, :] = r["outAnT"][i].astype(np.float32).T
        out[4 * L, sl, :] = r["outRdT"].astype(np.float32).T
    if bb is not None:
        out[3 * L:4 * L] = bb
    return out


_CACHE = {}


def _get_program(use_bias=False):
    key = ("nc", use_bias)
    if key not in _CACHE:
        _CACHE[key] = build_program(use_bias)
    return _CACHE[key]


def kernel(**inputs):
    use_bias = bool(np.any(inputs["b_ff"]) or np.any(inputs["b_fb"])
                    or np.any(inputs["b_out"]))
    nc = _get_program(use_bias)
    in_maps = make_in_maps(**inputs)
    bb = host_bb(inputs["b"], inputs["spikes_h"])
    res = run_bass_kernel_spmd(nc, in_maps, core_ids=list(range(NCORES)))
    return assemble_output(res.results, bb)


# revision 7
# speedup vs baseline: 1.3538x; 1.0299x over previous
"""EnergySNN single-step kernel for Trainium2, 8-core data parallel.

Reference computation (per batch row, D=512, L=3 layers):
    s = 0.5*x
    for i in 0..2:
        fb_in = spikes_h[i+1]            (i<2)   |  readout/||readout||  (i==2)
        ff = s @ W_ff[i].T + b_ff[i]
        fb = fb_in @ W_fb[i].T + b_fb[i]
        a_new = 0.9*dend[i] + 0.1*(ff+fb)
        sm    = 0.9*soma[i]*(1-spikes_h[i]) + 0.1*a_new
        bb    = 0.96*b[i] + 0.04*spikes_h[i]
        spk   = (sm - (0.1 + 1.8*bb)) > 0
        s = spk
    readout_new = 0.9*readout + s @ W_out.T + b_out
    out = [sm(3), spk(3), a_new(3), bb(3), readout_new(1)]  -> [13, B, D]

Strategy: pure data parallel over batch (8192 -> 8 x 1024), transposed
[D, B_local] device layout so matmul rhs (contraction over D on partitions)
and elementwise state updates share one layout.

Numerics: rhs planes (0.5*x, spike history, normalized readout), the spike
outputs AND the W_ff/W_fb weights are fp8e4m3 (spikes are exactly 0/1 in
fp8); ff/fb matmuls run in DoubleRow perf mode (2 fp8 weights per PE cell,
one instruction per contraction PAIR). Weights stay unfolded so they don't
sink into fp8's denormal range; the 0.1/0.9 prefactors move into the
identity-matmul scale (10*I), the activation-copy scale (0.1) and the sm
STT scale (0.01). States are fp16/bf16, W_out stays bf16 for readout
accuracy. Elementwise algebra is folded on the host into fused planes so
the device does only:
    psum  = 10*dendm*I + ff + fb          (identity matmul folds the dendrite)
    a_new = 0.1*psum                       (scalar engine copy, f16 out)
    sm    = 0.01*psum + somam              (one DVE op; somam=0.9*soma*(1-s))
    spk   = (13.889*sm) > tbps             (one DVE op; tbps=(0.1+1.728b)/0.072+s)
The bb output plane (0.96*b + 0.04*s) needs no matmul and is computed on the
host. Total relative error ~6e-3 (tolerance 2e-2).

All load-side arrays are pre-swizzled on the host into the exact SBUF tile
layout ([128 partitions, chunks, freedim], contiguous per partition) so every
load DMA is a plain linear copy (large merged packets, cheap descriptor
generation). All loads go on the sync HWDGE queue in strict consumption
order -- DMA queues fair-share HBM by packet count, so a single ordered
queue is the only real priority mechanism. All stores go on the scalar
HWDGE queue (gpsimd DMA is the slow software-DGE path; never use it).
Each layer's spike outputs land in one wide [P, KC, NW] tile so the next
layer's ff rhs can form the 3D k-pair access patterns DoubleRow needs.
"""

import numpy as np
import sys

sys.path.insert(0, "/opt/trn_rl_repo")

import concourse.bass as bass
import concourse.bacc as bacc
import concourse.mybir as mybir
from concourse import tile
from concourse.bass_utils import run_bass_kernel_spmd

F32 = mybir.dt.float32
BF16 = mybir.dt.bfloat16
F16 = mybir.dt.float16
F8 = mybir.dt.float8e4
NP_BF16 = mybir.dt.np(BF16)
NP_F8 = mybir.dt.np(F8)
OP = mybir.AluOpType
AF = mybir.ActivationFunctionType
DR = mybir.MatmulPerfMode.DoubleRow

# Problem constants (hardcoded per contract)
B = 8192
D = 512
L = 3
NCORES = 8
BL = B // NCORES          # 1024 batch rows per core
P = 128                   # partitions
KC = D // P               # 4 contraction chunks
MC = D // P               # 4 output-d chunks
NW = BL                   # full local batch as one free-dim tile
NH = 512                  # matmul free-dim (one PSUM bank of f32)
NCH = NW // NH            # 2 matmul half-groups per psum tile

ALPHA_M = np.float32(0.9)
ALPHA_A = np.float32(0.9)
RHO = np.float32(0.96)
BETA = np.float32(1.8)
B0 = np.float32(0.1)
ALPHA_OUT = np.float32(0.9)
EPS = np.float32(1e-12)
SC = np.float32(1.0) / (BETA * (np.float32(1.0) - RHO))   # 1/0.072

OUTPUT_NAMES = ["outSmT", "outAnT", "outSpkT", "outRdT"]


def build_program(use_bias=False):
    """Build the per-core SPMD Bass/Tile program."""
    nc = bacc.Bacc("TRN2", target_bir_lowering=False)

    # --- DRAM I/O (per-core shapes, host-preswizzled tile layouts) ---
    xT = nc.dram_tensor("xT", [P, KC, NW], F8, kind="ExternalInput")
    spk1T = nc.dram_tensor("spk1T", [P, KC, NW], F8, kind="ExternalInput")
    spk2T = nc.dram_tensor("spk2T", [P, KC, NW], F8, kind="ExternalInput")
    fbnT = nc.dram_tensor("fbnT", [P, KC, NW], F8, kind="ExternalInput")
    readT = nc.dram_tensor("readT", [P, MC * NW], F16, kind="ExternalInput")
    somamT = nc.dram_tensor("somamT", [L, P, MC * NW], F16, kind="ExternalInput")
    dendmT = nc.dram_tensor("dendmT", [L, P, MC * NW], BF16, kind="ExternalInput")
    tbpsT = nc.dram_tensor("tbpsT", [L, P, MC * NW], F16, kind="ExternalInput")
    wffT = nc.dram_tensor("wffT", [L, P, KC, D], F8, kind="ExternalInput")
    wfbT = nc.dram_tensor("wfbT", [L, P, KC, D], F8, kind="ExternalInput")
    woutT = nc.dram_tensor("woutT", [P, KC, D], BF16, kind="ExternalInput")
    identT = nc.dram_tensor("identT", [P, P], BF16, kind="ExternalInput")
    bcomb = nc.dram_tensor("bcomb", [L, 1, D], BF16, kind="ExternalInput")
    boutD = nc.dram_tensor("boutD", [1, D], BF16, kind="ExternalInput")
    # outputs (transposed [D, BL] world)
    outSmT = nc.dram_tensor("outSmT", [L, D, BL], F16, kind="ExternalOutput")
    outAnT = nc.dram_tensor("outAnT", [L, D, BL], F16, kind="ExternalOutput")
    outSpkT = nc.dram_tensor("outSpkT", [L, D, BL], F8, kind="ExternalOutput")
    outRdT = nc.dram_tensor("outRdT", [D, BL], F16, kind="ExternalOutput")

    with tile.TileContext(nc) as tc:
        with (
            tc.tile_pool(name="wpool", bufs=1) as wp,
            tc.tile_pool(name="spool", bufs=1) as sp,
            tc.tile_pool(name="ppool", bufs=1, space=bass.MemorySpace.PSUM) as pp,
        ):
            ident = wp.tile([P, P], BF16, tag="ident")
            nc.sync.dma_start(ident[:], identT[:, :])
            if use_bias:
                onesN = wp.tile([1, NH], BF16, tag="onesN")
                nc.vector.memset(onesN[:], 1.0)
                bc_sb = [wp.tile([1, D], BF16, tag=f"bc{i}", name=f"bc{i}")
                         for i in range(L)]
                bo_sb = wp.tile([1, D], BF16, tag="bo")
                for i in range(L):
                    nc.sync.dma_start(bc_sb[i][:], bcomb[i, :, :])
                nc.sync.dma_start(bo_sb[:], boutD[:, :])

            def wload(name, src, dt):
                t = wp.tile([P, KC, D], dt, tag="w", bufs=7, name=name)
                nc.sync.dma_start(t[:], src)
                return t

            def rload(name, src):
                t = wp.tile([P, KC, NW], F8, tag="r", bufs=4, name=name)
                nc.sync.dma_start(t[:], src)
                return t

            def sload(name, src, dt, tag):
                t = sp.tile([P, MC * NW], dt, tag=tag, bufs=3, name=name)
                nc.sync.dma_start(t[:], src)
                return t

            # ---- all loads on sync, strict consumption order ----
            w0ff = wload("w0ff", wffT[0], F8)
            w0fb = wload("w0fb", wfbT[0], F8)
            x_sb = rload("x_sb", xT[:, :, :])
            s1_sb = rload("s1_sb", spk1T[:, :, :])
            dendm0 = sload("dendm0", dendmT[0], BF16, "dendm")
            somam0 = sload("somam0", somamT[0], F16, "somam")
            tbps0 = sload("tbps0", tbpsT[0], F16, "tbps")
            w1ff = wload("w1ff", wffT[1], F8)
            w1fb = wload("w1fb", wfbT[1], F8)
            s2_sb = rload("s2_sb", spk2T[:, :, :])
            dendm1 = sload("dendm1", dendmT[1], BF16, "dendm")
            somam1 = sload("somam1", somamT[1], F16, "somam")
            tbps1 = sload("tbps1", tbpsT[1], F16, "tbps")
            w2ff = wload("w2ff", wffT[2], F8)
            w2fb = wload("w2fb", wfbT[2], F8)
            fbn_sb = rload("fbn_sb", fbnT[:, :, :])
            dendm2 = sload("dendm2", dendmT[2], BF16, "dendm")
            somam2 = sload("somam2", somamT[2], F16, "somam")
            tbps2 = sload("tbps2", tbpsT[2], F16, "tbps")
            read_sb = sload("read_sb", readT[:, :], F16, "read")
            wout_sb = wload("wout", woutT[:, :, :], BF16)

            wgt = {0: (w0ff, w0fb), 1: (w1ff, w1fb), 2: (w2ff, w2fb)}
            stt = {0: (dendm0, somam0, tbps0), 1: (dendm1, somam1, tbps1),
                   2: (dendm2, somam2, tbps2)}
            rhs_fb_t = {0: s1_sb, 1: s2_sb, 2: fbn_sb}

            # ---- layer loop ----
            rhs_ff = x_sb
            for i in range(L):
                ff_w, fb_w = wgt[i]
                dm, so, tb = stt[i]
                fb_t = rhs_fb_t[i]
                spkw = sp.tile([P, KC, NW], F8, tag="spkw", bufs=3,
                               name=f"spkw{i}")
                for m in range(MC):
                    msl = slice(m * P, (m + 1) * P)
                    ps = pp.tile([P, NW], F32, tag="mm", bufs=3)
                    for n in range(NCH):
                        psn = ps[:, n * NH:(n + 1) * NH]
                        nsl = slice(n * NH, (n + 1) * NH)
                        dsl = dm[:, m * NW + n * NH: m * NW + n * NH + NH]
                        nc.tensor.matmul(psn, ident[:], dsl, start=True,
                                         stop=False)
                        for kp in range(0, KC, 2):
                            nc.tensor.matmul(psn, fb_w[:, kp:kp + 2, msl],
                                             fb_t[:, kp:kp + 2, nsl],
                                             start=False, stop=False,
                                             perf_mode=DR)
                        for kp in range(0, KC, 2):
                            last = (kp + 2 >= KC) and not use_bias
                            nc.tensor.matmul(psn, ff_w[:, kp:kp + 2, msl],
                                             rhs_ff[:, kp:kp + 2, nsl],
                                             start=False, stop=last,
                                             perf_mode=DR)
                        if use_bias:
                            nc.tensor.matmul(psn, bc_sb[i][0:1, msl],
                                             onesN[0:1, :], start=False,
                                             stop=True)
                    wsl2 = slice(m * NW, (m + 1) * NW)
                    anew = sp.tile([P, NW], F16, tag="anew", bufs=4)
                    nc.scalar.activation(anew[:], ps[:], AF.Copy, scale=0.1)
                    sm = sp.tile([P, NW], F16, tag="sm", bufs=4)
                    nc.vector.scalar_tensor_tensor(
                        sm[:], ps[:], 0.01, so[:, wsl2], OP.mult, OP.add)
                    nc.vector.scalar_tensor_tensor(
                        spkw[:, m, :], sm[:], float(SC), tb[:, wsl2],
                        OP.mult, OP.is_gt)
                    nc.scalar.dma_start(outAnT[i, msl, :], anew[:])
                    nc.scalar.dma_start(outSmT[i, msl, :], sm[:])
                    nc.scalar.dma_start(outSpkT[i, msl, :], spkw[:, m, :])
                rhs_ff = spkw

            # ---- readout: 0.9*readout + spk2 @ W_out.T + b_out ----
            for m in range(MC):
                msl = slice(m * P, (m + 1) * P)
                psr = pp.tile([P, NW], F32, tag="mm", bufs=3)
                for n in range(NCH):
                    psn = psr[:, n * NH:(n + 1) * NH]
                    nsl = slice(n * NH, (n + 1) * NH)
                    for k in range(KC):
                        last = (k == KC - 1) and not use_bias
                        nc.tensor.matmul(psn, wout_sb[:, k, msl],
                                         rhs_ff[:, k, nsl],
                                         start=(k == 0), stop=last)
                    if use_bias:
                        nc.tensor.matmul(psn, bo_sb[0:1, msl],
                                         onesN[0:1, :], start=False, stop=True)
                rn = sp.tile([P, NW], F16, tag="rn", bufs=2)
                nc.vector.scalar_tensor_tensor(
                    rn[:], read_sb[:, m * NW:(m + 1) * NW], float(ALPHA_OUT),
                    psr[:], OP.mult, OP.add)
                nc.scalar.dma_start(outRdT[msl, :], rn[:])

    nc.compile()
    return nc


def _swz(a2d):
    """[D, X] -> [P, D//P, X] SBUF tile layout, contiguous per partition."""
    Dd, X = a2d.shape
    k = Dd // P
    return np.ascontiguousarray(a2d.reshape(k, P, X).transpose(1, 0, 2))


def make_in_maps(x, soma, spikes_h, dendrites, b, readout,
                 W_ff, b_ff, W_fb, b_fb, W_out, b_out):
    """Shard + transpose inputs; fold elementwise algebra into fused planes."""
    f32 = np.float32
    f16 = np.float16
    x = np.asarray(x, f32)
    soma = np.asarray(soma, f32)
    spikes_h = np.asarray(spikes_h, f32)
    dendrites = np.asarray(dendrites, f32)
    b = np.asarray(b, f32)
    readout = np.asarray(readout, f32)
    W_ff = np.asarray(W_ff, f32)
    b_ff = np.asarray(b_ff, f32)
    W_fb = np.asarray(W_fb, f32)
    b_fb = np.asarray(b_fb, f32)
    W_out = np.asarray(W_out, f32)
    b_out = np.asarray(b_out, f32)

    # weights transposed, UNFOLDED (fp8 can't hold 0.1-scaled values well)
    wffT = np.stack([_swz(W_ff[i].T.astype(NP_F8)) for i in range(L)])
    wfbT = np.stack([_swz(W_fb[i].T.astype(NP_F8)) for i in range(L)])
    woutT = _swz(W_out.T.astype(NP_BF16))
    identA = (np.float32(10.0) * np.eye(P, dtype=f32)).astype(NP_BF16)
    bcombA = (b_ff + b_fb).reshape(L, 1, D).astype(NP_BF16)
    boutA = b_out.reshape(1, D).astype(NP_BF16)

    # fused state planes (full batch, then shard)
    somam = (ALPHA_M * soma * (f32(1.0) - spikes_h)).astype(f16)
    dendm = (ALPHA_A * dendrites).astype(NP_BF16)
    tbps = (SC * (B0 + BETA * RHO * b) + spikes_h).astype(f16)
    nrm = np.maximum(np.sqrt(np.sum(readout * readout, axis=1, keepdims=True)),
                     EPS).astype(f32)
    fbn = (readout / nrm).astype(NP_F8)
    x16 = (f32(0.5) * x).astype(NP_F8)       # 0.5 input scale folded into x
    spk16 = spikes_h.astype(NP_F8)
    read16 = readout.astype(f16)

    in_maps = []
    for c in range(NCORES):
        sl = slice(c * BL, (c + 1) * BL)
        in_maps.append({
            "xT": _swz(x16[sl].T),
            "spk1T": _swz(spk16[1, sl].T),
            "spk2T": _swz(spk16[2, sl].T),
            "fbnT": _swz(fbn[sl].T),
            "readT": _swz(read16[sl].T).reshape(P, MC * NW),
            "somamT": np.stack([_swz(somam[i, sl].T).reshape(P, MC * NW)
                                for i in range(L)]),
            "dendmT": np.stack([_swz(dendm[i, sl].T).reshape(P, MC * NW)
                                for i in range(L)]),
            "tbpsT": np.stack([_swz(tbps[i, sl].T).reshape(P, MC * NW)
                               for i in range(L)]),
            "wffT": wffT,
            "wfbT": wfbT,
            "woutT": woutT,
            "identT": identA,
            "bcomb": bcombA,
            "boutD": boutA,
        })
    return in_maps


def host_bb(b, spikes_h):
    """bb = 0.96*b + 0.04*s needs no matmul -> exact f32 on the host."""
    return (RHO * np.asarray(b, np.float32)
            + (np.float32(1.0) - RHO) * np.asarray(spikes_h, np.float32))


def assemble_output(results, bb=None):
    """per-core 16-bit/fp8 outputs -> [13, B, D] f32 (+ host bb planes)."""
    out = np.empty((4 * L + 1, B, D), np.float32)
    for c in range(NCORES):
        sl = slice(c * BL, (c + 1) * BL)
        r = results[c]
        for i in range(L):
            out[i, sl, :] = r["outSmT"][i].astype(np.float32).T
            out[L + i, sl, :] = r["outSpkT"][i].astype(np.float32).T
            out[2 * L + i, sl, :] = r["outAnT"][i].astype(np.float32).T
        out[4 * L, sl, :] = r["outRdT"].astype(np.float32).T
    if bb is not None:
        out[3 * L:4 * L] = bb
    return out


_CACHE = {}


def _get_program(use_bias=False):
    key = ("nc", use_bias)
    if key not in _CACHE:
        _CACHE[key] = build_program(use_bias)
    return _CACHE[key]


def kernel(**inputs):
    use_bias = bool(np.any(inputs["b_ff"]) or np.any(inputs["b_fb"])
                    or np.any(inputs["b_out"]))
    nc = _get_program(use_bias)
    in_maps = make_in_maps(**inputs)
    bb = host_bb(inputs["b"], inputs["spikes_h"])
    res = run_bass_kernel_spmd(nc, in_maps, core_ids=list(range(NCORES)))
    return assemble_output(res.results, bb)


# revision 8
# speedup vs baseline: 1.4183x; 1.0477x over previous
"""EnergySNN single-step kernel for Trainium2, 8-core data parallel.

Reference computation (per batch row, D=512, L=3 layers):
    s = 0.5*x
    for i in 0..2:
        fb_in = spikes_h[i+1]            (i<2)   |  readout/||readout||  (i==2)
        ff = s @ W_ff[i].T + b_ff[i]
        fb = fb_in @ W_fb[i].T + b_fb[i]
        a_new = 0.9*dend[i] + 0.1*(ff+fb)
        sm    = 0.9*soma[i]*(1-spikes_h[i]) + 0.1*a_new
        bb    = 0.96*b[i] + 0.04*spikes_h[i]
        spk   = (sm - (0.1 + 1.8*bb)) > 0
        s = spk
    readout_new = 0.9*readout + s @ W_out.T + b_out
    out = [sm(3), spk(3), a_new(3), bb(3), readout_new(1)]  -> [13, B, D]

Strategy: pure data parallel over batch (8192 -> 8 x 1024), transposed
[D, B_local] device layout so matmul rhs (contraction over D on partitions)
and elementwise state updates share one layout.

Numerics: rhs planes (0.5*x, spike history, normalized readout), the spike
outputs AND the W_ff/W_fb weights are fp8e4m3 (spikes are exactly 0/1 in
fp8); ff/fb matmuls run in DoubleRow perf mode (2 fp8 weights per PE cell,
one instruction per contraction PAIR). Weights stay unfolded so they don't
sink into fp8's denormal range; the 0.1/0.9 prefactors move into the
identity-matmul scale (10*I), the activation-copy scale (0.1) and the sm
STT scale (0.01). States are fp16/bf16, W_out stays bf16 for readout
accuracy. Elementwise algebra is folded on the host into fused planes so
the device does only:
    psum  = 10*dendm*I + ff + fb          (identity matmul folds the dendrite)
    a_new = 0.1*psum                       (scalar engine copy, f16 out)
    sm    = 0.01*psum + somam              (one DVE op; somam=0.9*soma*(1-s))
    spk   = (13.889*sm) > tbps             (one DVE op; tbps=(0.1+1.728b)/0.072+s)
The bb output plane (0.96*b + 0.04*s) needs no matmul and is computed on the
host. Total relative error ~6e-3 (tolerance 2e-2).

All load-side arrays are pre-swizzled on the host into the exact SBUF tile
layout ([128 partitions, chunks, freedim], contiguous per partition) so every
load DMA is a plain linear copy (large merged packets, cheap descriptor
generation). All loads go on the sync HWDGE queue in strict consumption
order -- DMA queues fair-share HBM by packet count, so a single ordered
queue is the only real priority mechanism. All stores go on the scalar
HWDGE queue (gpsimd DMA is the slow software-DGE path; never use it).
Each layer's spike outputs land in one wide [P, KC, NW] tile so the next
layer's ff rhs can form the 3D k-pair access patterns DoubleRow needs.
"""

import numpy as np
import sys

sys.path.insert(0, "/opt/trn_rl_repo")

import concourse.bass as bass
import concourse.bacc as bacc
import concourse.mybir as mybir
from concourse import tile
from concourse.bass_utils import run_bass_kernel_spmd

F32 = mybir.dt.float32
BF16 = mybir.dt.bfloat16
F16 = mybir.dt.float16
F8 = mybir.dt.float8e4
NP_BF16 = mybir.dt.np(BF16)
NP_F8 = mybir.dt.np(F8)
OP = mybir.AluOpType
AF = mybir.ActivationFunctionType
DR = mybir.MatmulPerfMode.DoubleRow

# Problem constants (hardcoded per contract)
B = 8192
D = 512
L = 3
NCORES = 8
BL = B // NCORES          # 1024 batch rows per core
P = 128                   # partitions
KC = D // P               # 4 contraction chunks
MC = D // P               # 4 output-d chunks
NW = BL                   # full local batch as one free-dim tile
NH = 512                  # matmul free-dim (one PSUM bank of f32)
NCH = NW // NH            # 2 matmul half-groups per psum tile

ALPHA_M = np.float32(0.9)
ALPHA_A = np.float32(0.9)
RHO = np.float32(0.96)
BETA = np.float32(1.8)
B0 = np.float32(0.1)
ALPHA_OUT = np.float32(0.9)
EPS = np.float32(1e-12)
SC = np.float32(1.0) / (BETA * (np.float32(1.0) - RHO))   # 1/0.072

OUTPUT_NAMES = ["outSmT", "outAnT", "outSpkT", "outRdT"]


def build_program(use_bias=False):
    """Build the per-core SPMD Bass/Tile program."""
    nc = bacc.Bacc("TRN2", target_bir_lowering=False)

    # --- DRAM I/O (per-core shapes, host-preswizzled tile layouts) ---
    xT = nc.dram_tensor("xT", [P, KC, NW], F8, kind="ExternalInput")
    spk1T = nc.dram_tensor("spk1T", [P, KC, NW], F8, kind="ExternalInput")
    spk2T = nc.dram_tensor("spk2T", [P, KC, NW], F8, kind="ExternalInput")
    fbnT = nc.dram_tensor("fbnT", [P, KC, NW], F8, kind="ExternalInput")
    readT = nc.dram_tensor("readT", [P, MC * NW], F16, kind="ExternalInput")
    somamT = nc.dram_tensor("somamT", [L, P, MC * NW], F16, kind="ExternalInput")
    dendmT = nc.dram_tensor("dendmT", [L, P, MC * NW], BF16, kind="ExternalInput")
    tbpsT = nc.dram_tensor("tbpsT", [L, P, MC * NW], F16, kind="ExternalInput")
    wffT = nc.dram_tensor("wffT", [L, P, KC, D], F8, kind="ExternalInput")
    wfbT = nc.dram_tensor("wfbT", [L, P, KC, D], F8, kind="ExternalInput")
    woutT = nc.dram_tensor("woutT", [P, KC, D], BF16, kind="ExternalInput")
    identT = nc.dram_tensor("identT", [P, P], BF16, kind="ExternalInput")
    bcomb = nc.dram_tensor("bcomb", [L, 1, D], BF16, kind="ExternalInput")
    boutD = nc.dram_tensor("boutD", [1, D], BF16, kind="ExternalInput")
    # outputs (transposed [D, BL] world)
    outSmT = nc.dram_tensor("outSmT", [L, P, MC * NW], F16, kind="ExternalOutput")
    outAnT = nc.dram_tensor("outAnT", [L, P, MC * NW], F16, kind="ExternalOutput")
    outSpkT = nc.dram_tensor("outSpkT", [L, P, MC * NW], F8, kind="ExternalOutput")
    outRdT = nc.dram_tensor("outRdT", [P, MC * NW], F16, kind="ExternalOutput")

    with tile.TileContext(nc) as tc:
        with (
            tc.tile_pool(name="wpool", bufs=1) as wp,
            tc.tile_pool(name="spool", bufs=1) as sp,
            tc.tile_pool(name="ppool", bufs=1, space=bass.MemorySpace.PSUM) as pp,
        ):
            ident = wp.tile([P, P], BF16, tag="ident")
            nc.sync.dma_start(ident[:], identT[:, :])
            if use_bias:
                onesN = wp.tile([1, NH], BF16, tag="onesN")
                nc.vector.memset(onesN[:], 1.0)
                bc_sb = [wp.tile([1, D], BF16, tag=f"bc{i}", name=f"bc{i}")
                         for i in range(L)]
                bo_sb = wp.tile([1, D], BF16, tag="bo")
                for i in range(L):
                    nc.sync.dma_start(bc_sb[i][:], bcomb[i, :, :])
                nc.sync.dma_start(bo_sb[:], boutD[:, :])

            def wload(name, src, dt):
                t = wp.tile([P, KC, D], dt, tag="w", bufs=7, name=name)
                nc.sync.dma_start(t[:], src)
                return t

            def rload(name, src):
                t = wp.tile([P, KC, NW], F8, tag="r", bufs=4, name=name)
                nc.sync.dma_start(t[:], src)
                return t

            def sload(name, src, dt, tag):
                t = sp.tile([P, MC * NW], dt, tag=tag, bufs=3, name=name)
                nc.sync.dma_start(t[:], src)
                return t

            # ---- all loads on sync, strict consumption order ----
            w0ff = wload("w0ff", wffT[0], F8)
            w0fb = wload("w0fb", wfbT[0], F8)
            dendm0 = sload("dendm0", dendmT[0], BF16, "dendm")
            x_sb = rload("x_sb", xT[:, :, :])
            s1_sb = rload("s1_sb", spk1T[:, :, :])
            somam0 = sload("somam0", somamT[0], F16, "somam")
            tbps0 = sload("tbps0", tbpsT[0], F16, "tbps")
            w1ff = wload("w1ff", wffT[1], F8)
            w1fb = wload("w1fb", wfbT[1], F8)
            s2_sb = rload("s2_sb", spk2T[:, :, :])
            dendm1 = sload("dendm1", dendmT[1], BF16, "dendm")
            somam1 = sload("somam1", somamT[1], F16, "somam")
            tbps1 = sload("tbps1", tbpsT[1], F16, "tbps")
            w2ff = wload("w2ff", wffT[2], F8)
            w2fb = wload("w2fb", wfbT[2], F8)
            fbn_sb = rload("fbn_sb", fbnT[:, :, :])
            dendm2 = sload("dendm2", dendmT[2], BF16, "dendm")
            somam2 = sload("somam2", somamT[2], F16, "somam")
            tbps2 = sload("tbps2", tbpsT[2], F16, "tbps")
            read_sb = sload("read_sb", readT[:, :], F16, "read")
            wout_sb = wload("wout", woutT[:, :, :], BF16)

            wgt = {0: (w0ff, w0fb), 1: (w1ff, w1fb), 2: (w2ff, w2fb)}
            stt = {0: (dendm0, somam0, tbps0), 1: (dendm1, somam1, tbps1),
                   2: (dendm2, somam2, tbps2)}
            rhs_fb_t = {0: s1_sb, 1: s2_sb, 2: fbn_sb}

            # ---- layer loop ----
            rhs_ff = x_sb
            for i in range(L):
                ff_w, fb_w = wgt[i]
                dm, so, tb = stt[i]
                fb_t = rhs_fb_t[i]
                spkw = sp.tile([P, KC, NW], F8, tag="spkw", bufs=3,
                               name=f"spkw{i}")
                smw = sp.tile([P, MC, NW], F16, tag="smw", bufs=2,
                              name=f"smw{i}")
                anw = sp.tile([P, MC, NW], F16, tag="anw", bufs=2,
                              name=f"anw{i}")
                for m in range(MC):
                    msl = slice(m * P, (m + 1) * P)
                    ps = pp.tile([P, NW], F32, tag="mm", bufs=3)
                    for n in range(NCH):
                        psn = ps[:, n * NH:(n + 1) * NH]
                        nsl = slice(n * NH, (n + 1) * NH)
                        dsl = dm[:, m * NW + n * NH: m * NW + n * NH + NH]
                        nc.tensor.matmul(psn, ident[:], dsl, start=True,
                                         stop=False)
                        for kp in range(0, KC, 2):
                            nc.tensor.matmul(psn, fb_w[:, kp:kp + 2, msl],
                                             fb_t[:, kp:kp + 2, nsl],
                                             start=False, stop=False,
                                             perf_mode=DR)
                        for kp in range(0, KC, 2):
                            last = (kp + 2 >= KC) and not use_bias
                            nc.tensor.matmul(psn, ff_w[:, kp:kp + 2, msl],
                                             rhs_ff[:, kp:kp + 2, nsl],
                                             start=False, stop=last,
                                             perf_mode=DR)
                        if use_bias:
                            nc.tensor.matmul(psn, bc_sb[i][0:1, msl],
                                             onesN[0:1, :], start=False,
                                             stop=True)
                    wsl2 = slice(m * NW, (m + 1) * NW)
                    nc.scalar.activation(anw[:, m, :], ps[:], AF.Copy,
                                         scale=0.1)
                    nc.vector.scalar_tensor_tensor(
                        smw[:, m, :], ps[:], 0.01, so[:, wsl2],
                        OP.mult, OP.add)
                    nc.vector.scalar_tensor_tensor(
                        spkw[:, m, :], smw[:, m, :], float(SC), tb[:, wsl2],
                        OP.mult, OP.is_gt)
                # one wide store per plane, on the SAME sync queue as the
                # loads: all load descriptors sit ahead in the ring, so loads
                # get strict priority and stores drain afterwards.
                nc.sync.dma_start(outAnT[i], anw[:].rearrange("p k n -> p (k n)"))
                nc.sync.dma_start(outSmT[i], smw[:].rearrange("p k n -> p (k n)"))
                nc.sync.dma_start(outSpkT[i], spkw[:].rearrange(
                    "p k n -> p (k n)"))
                rhs_ff = spkw

            # ---- readout: 0.9*readout + spk2 @ W_out.T + b_out ----
            rnw = sp.tile([P, MC, NW], F16, tag="rnw", bufs=1)
            for m in range(MC):
                msl = slice(m * P, (m + 1) * P)
                psr = pp.tile([P, NW], F32, tag="mm", bufs=3)
                for n in range(NCH):
                    psn = psr[:, n * NH:(n + 1) * NH]
                    nsl = slice(n * NH, (n + 1) * NH)
                    for k in range(KC):
                        last = (k == KC - 1) and not use_bias
                        nc.tensor.matmul(psn, wout_sb[:, k, msl],
                                         rhs_ff[:, k, nsl],
                                         start=(k == 0), stop=last)
                    if use_bias:
                        nc.tensor.matmul(psn, bo_sb[0:1, msl],
                                         onesN[0:1, :], start=False, stop=True)
                nc.vector.scalar_tensor_tensor(
                    rnw[:, m, :], read_sb[:, m * NW:(m + 1) * NW],
                    float(ALPHA_OUT), psr[:], OP.mult, OP.add)
            nc.sync.dma_start(outRdT[:, :], rnw[:].rearrange(
                "p k n -> p (k n)"))

    nc.compile()
    return nc


def _swz(a2d):
    """[D, X] -> [P, D//P, X] SBUF tile layout, contiguous per partition."""
    Dd, X = a2d.shape
    k = Dd // P
    return np.ascontiguousarray(a2d.reshape(k, P, X).transpose(1, 0, 2))


def make_in_maps(x, soma, spikes_h, dendrites, b, readout,
                 W_ff, b_ff, W_fb, b_fb, W_out, b_out):
    """Shard + transpose inputs; fold elementwise algebra into fused planes."""
    f32 = np.float32
    f16 = np.float16
    x = np.asarray(x, f32)
    soma = np.asarray(soma, f32)
    spikes_h = np.asarray(spikes_h, f32)
    dendrites = np.asarray(dendrites, f32)
    b = np.asarray(b, f32)
    readout = np.asarray(readout, f32)
    W_ff = np.asarray(W_ff, f32)
    b_ff = np.asarray(b_ff, f32)
    W_fb = np.asarray(W_fb, f32)
    b_fb = np.asarray(b_fb, f32)
    W_out = np.asarray(W_out, f32)
    b_out = np.asarray(b_out, f32)

    # weights transposed, UNFOLDED (fp8 can't hold 0.1-scaled values well)
    wffT = np.stack([_swz(W_ff[i].T.astype(NP_F8)) for i in range(L)])
    wfbT = np.stack([_swz(W_fb[i].T.astype(NP_F8)) for i in range(L)])
    woutT = _swz(W_out.T.astype(NP_BF16))
    identA = (np.float32(10.0) * np.eye(P, dtype=f32)).astype(NP_BF16)
    bcombA = (b_ff + b_fb).reshape(L, 1, D).astype(NP_BF16)
    boutA = b_out.reshape(1, D).astype(NP_BF16)

    # fused state planes (full batch, then shard)
    somam = (ALPHA_M * soma * (f32(1.0) - spikes_h)).astype(f16)
    dendm = (ALPHA_A * dendrites).astype(NP_BF16)
    tbps = (SC * (B0 + BETA * RHO * b) + spikes_h).astype(f16)
    nrm = np.maximum(np.sqrt(np.sum(readout * readout, axis=1, keepdims=True)),
                     EPS).astype(f32)
    fbn = (readout / nrm).astype(NP_F8)
    x16 = (f32(0.5) * x).astype(NP_F8)       # 0.5 input scale folded into x
    spk16 = spikes_h.astype(NP_F8)
    read16 = readout.astype(f16)

    in_maps = []
    for c in range(NCORES):
        sl = slice(c * BL, (c + 1) * BL)
        in_maps.append({
            "xT": _swz(x16[sl].T),
            "spk1T": _swz(spk16[1, sl].T),
            "spk2T": _swz(spk16[2, sl].T),
            "fbnT": _swz(fbn[sl].T),
            "readT": _swz(read16[sl].T).reshape(P, MC * NW),
            "somamT": np.stack([_swz(somam[i, sl].T).reshape(P, MC * NW)
                                for i in range(L)]),
            "dendmT": np.stack([_swz(dendm[i, sl].T).reshape(P, MC * NW)
                                for i in range(L)]),
            "tbpsT": np.stack([_swz(tbps[i, sl].T).reshape(P, MC * NW)
                               for i in range(L)]),
            "wffT": wffT,
            "wfbT": wfbT,
            "woutT": woutT,
            "identT": identA,
            "bcomb": bcombA,
            "boutD": boutA,
        })
    return in_maps


def host_bb(b, spikes_h):
    """bb = 0.96*b + 0.04*s needs no matmul -> exact f32 on the host."""
    return (RHO * np.asarray(b, np.float32)
            + (np.float32(1.0) - RHO) * np.asarray(spikes_h, np.float32))


def assemble_output(results, bb=None):
    """per-core 16-bit/fp8 outputs -> [13, B, D] f32 (+ host bb planes)."""
    def unswz(t):
        # [P, MC*NW] tile layout -> [BL, D]
        return np.ascontiguousarray(
            t.reshape(P, MC, NW).transpose(2, 1, 0).reshape(NW, D))

    out = np.empty((4 * L + 1, B, D), np.float32)
    for c in range(NCORES):
        sl = slice(c * BL, (c + 1) * BL)
        r = results[c]
        for i in range(L):
            out[i, sl, :] = unswz(r["outSmT"][i].astype(np.float32))
            out[L + i, sl, :] = unswz(r["outSpkT"][i].astype(np.float32))
            out[2 * L + i, sl, :] = unswz(r["outAnT"][i].astype(np.float32))
        out[4 * L, sl, :] = unswz(r["outRdT"].astype(np.float32))
    if bb is not None:
        out[3 * L:4 * L] = bb
    return out


_CACHE = {}


def _get_program(use_bias=False):
    key = ("nc", use_bias)
    if key not in _CACHE:
        _CACHE[key] = build_program(use_bias)
    return _CACHE[key]


def kernel(**inputs):
    use_bias = bool(np.any(inputs["b_ff"]) or np.any(inputs["b_fb"])
                    or np.any(inputs["b_out"]))
    nc = _get_program(use_bias)
    in_maps = make_in_maps(**inputs)
    bb = host_bb(inputs["b"], inputs["spikes_h"])
    res = run_bass_kernel_spmd(nc, in_maps, core_ids=list(range(NCORES)))
    return assemble_output(res.results, bb)
